# revision 5
# baseline (speedup 1.0000x reference)
"""MoE (top-2 of 8 experts) Trainium2 kernel v3, 8-core data-parallel.

v3 = v2 (host routing, indirect gather, PE transposes, scatter-add combine)
plus:
  - host token-shuffle: tokens are dealt round-robin by expert-selection
    pattern so every core sees near-identical per-expert counts
    (capacity sum 2112 vs 2304 unshuffled)
  - feature-major layer 2: eoT[d, C] = W2^T @ hTs per 128-row d-tile, cost
    proportional to C exactly (no 128-slot partition quantization), bias b2
    applied per-partition in the PSUM->SBUF copy, then PE transpose-out and
    per-partition gate multiply to build token-major rows for the scatter.
"""

import os
import sys

for _p in ("/root/.axon_site/_ro/trn_rl_repo", "/opt/trn_rl_repo"):
    if os.path.isdir(_p) and _p not in sys.path:
        sys.path.insert(0, _p)

import numpy as np
import ml_dtypes

import concourse.bass as bass
import concourse.bacc as bacc
import concourse.tile as tile
from concourse import mybir
from concourse.bass_utils import run_bass_kernel_spmd

F32 = mybir.dt.float32
BF16 = mybir.dt.bfloat16
I32 = mybir.dt.int32
AF = mybir.ActivationFunctionType
ALU = mybir.AluOpType
BFNP = ml_dtypes.bfloat16

D = 1024      # in_features
H = 4096      # hidden
E = 8         # experts
N_CORES = 8
T = 1024      # tokens per core
ND = D // 128   # feature tiles (8)
NH = H // 128   # hidden tiles (32)

REPS = 1   # device-side repeat loop (timing only; >1 wraps body in For_i)


def host_route(x, Wg, bg):
    """Host fp32 routing + load-balancing token shuffle.

    Returns (chunks, token_lists, metas):
      chunks: tuple of (e, C) in processing order (C descending), C mult of 16
      token_lists[c]: global token ids assigned to core c (len T, local order)
      metas[c]: idx/g/slot arrays [128, NJT]
    """
    xt = np.asarray(x, np.float32).reshape(-1, D)
    logits = xt @ np.asarray(Wg, np.float32) + np.asarray(bg, np.float32).reshape(-1)
    srt2 = np.sort(logits, -1)[:, -2:-1]
    sel = logits >= srt2                      # [N, E] top-2 membership
    m = logits.max(-1, keepdims=True)
    p = np.exp(logits - m)
    p /= p.sum(-1, keepdims=True)
    gates = (p * sel).astype(np.float32)      # [N, E]
    N = xt.shape[0]

    # deal tokens round-robin within identical selection patterns
    pair_id = sel @ (1 << np.arange(E))
    order = np.argsort(pair_id, kind="stable")
    assign = np.empty(N, np.int64)
    assign[order] = np.arange(N) % N_CORES
    token_lists = [np.nonzero(assign == c)[0] for c in range(N_CORES)]

    counts = np.stack(
        [sel[token_lists[c]].sum(axis=0) for c in range(N_CORES)]
    )                                          # [cores, E]
    caps = counts.max(axis=0)
    caps = ((caps + 7) // 8) * 8
    chunk_order = np.argsort(-caps, kind="stable")
    chunks = tuple((int(e), int(caps[e])) for e in chunk_order)
    jts = [(c + 127) // 128 for (_, c) in chunks]
    njt = sum(jts)

    metas = []
    for c in range(N_CORES):
        toks_c = token_lists[c]
        selc = sel[toks_c]
        gc = gates[toks_c]
        idx = np.zeros((128, njt), np.int32)
        gg = np.zeros((128, njt), np.float32)
        slot = np.full((128, njt), 2 * T, np.int32)   # OOB sentinel (dropped)
        base = 0
        for (e, C), jt in zip(chunks, jts):
            loc = np.nonzero(selc[:, e])[0]           # local slot ids
            n = len(loc)
            assert n <= C
            s = np.arange(n)
            cols = base + s // 128
            rows = s % 128
            idx[rows, cols] = loc
            gg[rows, cols] = gc[loc, e]
            slot[rows, cols] = loc
            base += jt
        metas.append({"idx": idx, "g": gg, "slot": slot})
    return chunks, token_lists, metas


def _emit_moe(nc, tc, io, chunks):
    from contextlib import ExitStack

    jts = [(c + 127) // 128 for (_, c) in chunks]
    bases = np.cumsum([0] + jts).tolist()
    NJT = bases[-1]

    with ExitStack() as ctx:
        const = ctx.enter_context(tc.tile_pool(name="const", bufs=1))
        xgpool = ctx.enter_context(tc.tile_pool(name="xg", bufs=6))
        xtpool = ctx.enter_context(tc.tile_pool(name="xsT", bufs=2))
        hpool = ctx.enter_context(tc.tile_pool(name="hTs", bufs=2))
        w1pool = ctx.enter_context(tc.tile_pool(name="w1s", bufs=10))
        w2pool = ctx.enter_context(tc.tile_pool(name="w2s", bufs=8))
        ebpool = ctx.enter_context(tc.tile_pool(name="ebuf", bufs=4))
        epool = ctx.enter_context(tc.tile_pool(name="eo", bufs=6))
        zpool = ctx.enter_context(tc.tile_pool(name="zero", bufs=1))
        psum1 = ctx.enter_context(tc.tile_pool(name="psum1", bufs=2, space="PSUM"))
        psum2 = ctx.enter_context(tc.tile_pool(name="psum2", bufs=4, space="PSUM"))
        psumt = ctx.enter_context(tc.tile_pool(name="psumt", bufs=2, space="PSUM"))

        # ---- constants / metadata, packed into 2 DMAs (fewer small DMAs
        # at startup keeps the shared DMA-completion semaphore lanes clean,
        # so the chunk-0 gather/L1 aren't held behind unrelated transfers) ----
        meta_i = const.tile([128, 2 * NJT], I32)   # idx | slot
        nc.sync.dma_start(meta_i[:], io["meta_i"].ap())
        idx_t = meta_i[:, 0:NJT]
        slot_t = meta_i[:, NJT: 2 * NJT]

        ident = const.tile([128, 128], BF16)
        nc.sync.dma_start(ident[:], io["ident"].ap())

        # b1c [p, e*NH+i]=b1[e,i*128+p] | b2c [p, e*ND+dt]=b2[e,dt*128+p] | g
        meta_f = const.tile([128, E * NH + E * ND + NJT], F32)
        nc.sync.dma_start(meta_f[:], io["meta_f"].ap())
        b1c_t = meta_f[:, 0: E * NH]
        b2c_t = meta_f[:, E * NH: E * NH + E * ND]
        g_t = meta_f[:, E * NH + E * ND: E * NH + E * ND + NJT]

        out = io["out"].ap()   # [T, D] f32

        # zero the output; scatters accumulate into it
        ztile = zpool.tile([128, D], F32)
        nc.vector.memset(ztile[:], 0.0)
        for mt in range(T // 128):
            nc.scalar.dma_start(out[mt * 128:(mt + 1) * 128, :], ztile[:])

        x16 = io["x16"].ap()     # [T, D] bf16 (gather source)
        W1s_d = io["W1s"].ap()   # [E, NH, 128, ND*128] pre-swizzled lhsT blocks
        W2h = io["W2h"].ap()     # [E, 2, H, 512] bf16 (d-halves made contiguous)

        # gathers are emitted one chunk ahead of use (keeps DMA sem lanes
        # clean at startup, still hides gather latency under compute)
        all_xgs = {}

        def emit_gather(ci):
            e, C = chunks[ci]
            xgs = []
            for jt in range(jts[ci]):
                pj = min(128, C - jt * 128)
                xg = xgpool.tile([128, D], BF16, tag="xg", name=f"xg_{ci}_{jt}")
                nc.gpsimd.indirect_dma_start(
                    out=xg[:pj, :],
                    out_offset=None,
                    in_=x16,
                    in_offset=bass.IndirectOffsetOnAxis(
                        ap=idx_t[:pj, bases[ci] + jt: bases[ci] + jt + 1], axis=0
                    ),
                )
                xgs.append(xg)
            all_xgs[ci] = xgs

        emit_gather(0)

        for ci, (e, C) in enumerate(chunks):
            JT = jts[ci]
            base = bases[ci]
            xgs = all_xgs[ci]
            if ci + 1 < len(chunks):
                emit_gather(ci + 1)

            # ---- transpose to feature-major on the PE ----
            xsT = xtpool.tile([128, ND * C], BF16, tag="xsT")   # [p, dt*C + c]
            for jt in range(JT):
                pj = min(128, C - jt * 128)
                for dt in range(ND):
                    pst = psumt.tile(
                        [128, 128], BF16, tag="pst", name=f"pst_{ci}_{jt}_{dt}"
                    )
                    nc.tensor.transpose(
                        pst[:, :pj],
                        xgs[jt][:pj, dt * 128:(dt + 1) * 128],
                        ident[:pj, :pj],
                    )
                    nc.vector.tensor_copy(
                        xsT[:, dt * C + jt * 128: dt * C + jt * 128 + pj],
                        pst[:, :pj],
                    )

            # ---- layer 1: hTs[h, c] = gelu(W1^T @ xsT + b1) ----
            # W1 loaded in 2-ht pairs (512KB contiguous, 4KB/partition lines)
            hTs = hpool.tile([128, NH * C], BF16, tag="hTs")
            for hp in range(NH // 2):
                w1s = w1pool.tile([128, 2 * ND * 128], BF16, tag="w1s")
                nc.sync.dma_start(
                    w1s[:].rearrange("p (a q) -> p a q", a=2),
                    W1s_d[e, hp * 2: hp * 2 + 2].rearrange("a p q -> p a q"),
                )
                for a in range(2):
                    ht = hp * 2 + a
                    ps1 = psum1.tile([128, C], F32, tag="ps1", name=f"ps1_{ci}_{ht}")
                    for dt in range(ND):
                        nc.tensor.matmul(
                            ps1[:],
                            lhsT=w1s[:, a * 1024 + dt * 128: a * 1024 + (dt + 1) * 128],
                            rhs=xsT[:, dt * C:(dt + 1) * C],
                            start=(dt == 0),
                            stop=(dt == ND - 1),
                        )
                    nc.scalar.activation(
                        hTs[:, ht * C:(ht + 1) * C],
                        ps1[:],
                        AF.Gelu,
                        bias=b1c_t[:, e * NH + ht: e * NH + ht + 1],
                    )

            # ---- layer 2 feature-major: eoT[d, C] in two 4-tile passes ----
            eos = [
                epool.tile([128, D], F32, tag="eo", name=f"eo_{ci}_{jt}")
                for jt in range(JT)
            ]
            for half in range(2):
                pse = [
                    psum2.tile([128, C], F32, tag="ps2", name=f"ps2_{ci}_{half}_{dl}")
                    for dl in range(4)
                ]
                for hp in range(NH // 2):
                    # 2-ht pair per DMA: 256KB contiguous, 2KB/partition lines
                    w2s = w2pool.tile([128, 2 * 512], BF16, tag="w2s")
                    eng = nc.sync if hp % 4 == 0 else nc.scalar
                    eng.dma_start(
                        w2s[:].rearrange("p (a d) -> p a d", a=2),
                        W2h[e, half][hp * 256:(hp + 1) * 256, :].rearrange(
                            "(a p) d -> p a d", p=128
                        ),
                    )
                    for a in range(2):
                        ht = hp * 2 + a
                        for dl in range(4):
                            nc.tensor.matmul(
                                pse[dl][:],
                                lhsT=w2s[:, a * 512 + dl * 128: a * 512 + (dl + 1) * 128],
                                rhs=hTs[:, ht * C:(ht + 1) * C],
                                start=(ht == 0),
                                stop=(ht == NH - 1),
                            )
                for dl in range(4):
                    dt = half * 4 + dl
                    ebuf = ebpool.tile(
                        [128, C], BF16, tag="ebuf", name=f"eb_{ci}_{dt}"
                    )
                    nc.vector.tensor_scalar(
                        ebuf[:],
                        pse[dl][:],
                        b2c_t[:, e * ND + dt: e * ND + dt + 1],
                        None,
                        op0=ALU.add,
                    )
                    for jt in range(JT):
                        pj = min(128, C - jt * 128)
                        pst = psumt.tile(
                            [128, 128], BF16, tag="pst", name=f"psto_{ci}_{dt}_{jt}"
                        )
                        nc.tensor.transpose(
                            pst[:pj, :],
                            ebuf[:, jt * 128: jt * 128 + pj],
                            ident[:],
                        )
                        nc.vector.tensor_scalar_mul(
                            eos[jt][:pj, dt * 128:(dt + 1) * 128],
                            pst[:pj, :],
                            g_t[:pj, base + jt: base + jt + 1],
                        )

            # ---- scatter-add rows into out ----
            for jt in range(JT):
                pj = min(128, C - jt * 128)
                nc.gpsimd.indirect_dma_start(
                    out=out,
                    out_offset=bass.IndirectOffsetOnAxis(
                        ap=slot_t[:pj, base + jt: base + jt + 1], axis=0
                    ),
                    in_=eos[jt][:pj, :],
                    in_offset=None,
                    bounds_check=T - 1,
                    oob_is_err=False,
                    compute_op=ALU.add,
                )


def _build(chunks, njt):
    nc = bacc.Bacc(None, target_bir_lowering=False, debug=False, num_devices=N_CORES)
    io = {
        "x16": nc.declare_dram_parameter("x16", [T, D], BF16, isOutput=False),
        "W1s": nc.declare_dram_parameter(
            "W1s", [E, NH, 128, ND * 128], BF16, isOutput=False
        ),
        "W2h": nc.declare_dram_parameter("W2h", [E, 2, H, 512], BF16, isOutput=False),
        "ident": nc.declare_dram_parameter("ident", [128, 128], BF16, isOutput=False),
        "meta_i": nc.declare_dram_parameter(
            "meta_i", [128, 2 * njt], I32, isOutput=False
        ),
        "meta_f": nc.declare_dram_parameter(
            "meta_f", [128, E * NH + E * ND + njt], F32, isOutput=False
        ),
        "out": nc.declare_dram_parameter("out", [T, D], F32, isOutput=True),
    }
    with tile.TileContext(nc) as tc:
        if REPS > 1:
            with tc.For_i(0, REPS, 1):
                _emit_moe(nc, tc, io, chunks)
        else:
            _emit_moe(nc, tc, io, chunks)
    nc.compile()
    return nc


_CACHE = {}


def prep_inputs(x, Wg, bg, W1, b1, W2, b2):
    """Host-side shard + layout/dtype prep + routing. Returns
    (chunks, token_lists, in_maps)."""
    xt = np.ascontiguousarray(np.asarray(x, dtype=np.float32).reshape(-1, D))
    W1b = np.asarray(W1, dtype=np.float32).astype(BFNP)
    W2b = np.asarray(W2, dtype=np.float32).astype(BFNP)
    # d-halves contiguous: W2h[e, half, h, :] = W2[e, h, half*512:(half+1)*512]
    W2h = np.ascontiguousarray(
        W2b.reshape(E, H, 2, 512).transpose(0, 2, 1, 3)
    )
    b1c = np.ascontiguousarray(
        np.asarray(b1, dtype=np.float32).reshape(E, NH, 128).transpose(0, 2, 1)
    )
    b2c = np.ascontiguousarray(
        np.asarray(b2, dtype=np.float32).reshape(E, ND, 128).transpose(0, 2, 1)
    )

    # W1 swizzled so each (e, ht) slice is a contiguous [128, ND*128] lhsT
    # block: W1s[e, ht, p, dt*128 + j] = W1[e, dt*128 + p, ht*128 + j]
    W1s = np.ascontiguousarray(
        W1b.reshape(E, ND, 128, NH, 128).transpose(0, 3, 2, 1, 4).reshape(
            E, NH, 128, ND * 128
        )
    )

    chunks, token_lists, metas = host_route(x, Wg, bg)

    # packed layouts matching the device views:
    # b1c_p[p, e*NH+i] = b1[e, i*128+p]; b2c_p[p, e*ND+dt] = b2[e, dt*128+p]
    b1c_p = b1c.transpose(1, 0, 2).reshape(128, E * NH)
    b2c_p = b2c.transpose(1, 0, 2).reshape(128, E * ND)

    ident = np.eye(128, dtype=BFNP)
    in_maps = []
    for c in range(N_CORES):
        xs = xt[token_lists[c]]              # [T, D]
        meta_i = np.ascontiguousarray(
            np.concatenate([metas[c]["idx"], metas[c]["slot"]], axis=1)
        )
        meta_f = np.ascontiguousarray(
            np.concatenate([b1c_p, b2c_p, metas[c]["g"]], axis=1)
        )
        in_maps.append(
            {
                "x16": np.ascontiguousarray(xs.astype(BFNP)),
                "W1s": W1s,
                "W2h": W2h,
                "ident": ident,
                "meta_i": meta_i,
                "meta_f": meta_f,
            }
        )
    return chunks, token_lists, in_maps


def kernel(x, Wg, bg, W1, b1, W2, b2):
    B_, S_, D_ = x.shape
    chunks, token_lists, in_maps = prep_inputs(x, Wg, bg, W1, b1, W2, b2)
    key = chunks
    if key not in _CACHE:
        njt = sum((c + 127) // 128 for (_, c) in chunks)
        _CACHE[key] = _build(chunks, njt)
    nc = _CACHE[key]
    res = run_bass_kernel_spmd(nc, in_maps, list(range(N_CORES)))
    out = np.empty((B_ * S_, D_), np.float32)
    for c in range(N_CORES):
        out[token_lists[c]] = res.results[c]["out"]
    return out.reshape(B_, S_, D_)


if __name__ == "__main__":
    dat = np.load("/root/problem/_inputs.npz")
    inputs = {k: dat[k] for k in ("x", "Wg", "bg", "W1", "b1", "W2", "b2")}
    want = dat["ref"]
    got = kernel(**inputs)
    diff = np.abs(got - want)
    scale = np.abs(want).max()
    rel_fro = np.linalg.norm(diff) / np.linalg.norm(want)
    print(f"absmax err: {diff.max():.3e}  absmax/scale: {diff.max() / scale:.3e}  "
          f"rel_fro: {rel_fro:.3e}")
